# revision 7
# baseline (speedup 1.0000x reference)
"""Trainium2 Bass kernel for nn_ButterflyModule (8 stacked butterfly layers).

Math: each layer applies 64 disjoint Givens rotations over feature pairs
(gather via indices_in, scatter via idx_out). Every layer is therefore a
linear map A_l on the 128-dim feature axis, and the whole module collapses
into a single 128x128 matrix M = A_7 @ ... @ A_0 (2 nonzeros per row when
idx_out == indices_in, which setup_inputs always produces). M is composed
on host in float64 from the tiny angles/index inputs; the 256 MB `data`
tensor is processed on-device as out = data @ M.T.

Distribution: pure data-parallel over 8 NeuronCores — each core gets a
[65536, 128] batch shard, uploaded transposed as xt[128 feat, 65536 rows]
so the feature axis sits on SBUF partitions. The device kernel streams
row-chunks through the PE array with M.T as the stationary operand:

    DMA in [128, CH] -> matmul (lhsT = M.T, N=512 slices) -> PSUM
      -> DVE copy -> SBUF -> DMA out [128, CH]

which is purely HBM-bandwidth-bound (~64 MB of DRAM traffic per core).
"""

import numpy as np

B = 524288          # batch rows
F = 128             # feature dim
NUM_CORES = 8
R = B // NUM_CORES  # rows per core
CH = 8192           # rows (free-dim columns) per DMA chunk
NMM = 512           # matmul moving-operand free dim (1 PSUM bank of fp32)


def _build_nc(r=R, ch=CH):
    import concourse.bacc as bacc
    import concourse.mybir as mybir
    from concourse.tile import TileContext

    # Bacc (not raw Bass): its compile() runs move_matmul_waits_to_ldweights
    # + generate_event_semaphores, which split multi-semaphore waits to the
    # 1-wait-per-instruction hardware limit (walrus rejects them otherwise).
    nc = bacc.Bacc()
    xt = nc.dram_tensor("xt", [F, r], mybir.dt.float32, kind="ExternalInput")
    mt = nc.dram_tensor("mt", [F, F], mybir.dt.float32, kind="ExternalInput")
    ot = nc.dram_tensor("ot", [F, r], mybir.dt.float32, kind="ExternalOutput")

    n_chunks = r // ch
    assert n_chunks * ch == r and ch % NMM == 0

    with TileContext(nc) as tc:
        with (
            tc.tile_pool(name="consts", bufs=1) as cpool,
            tc.tile_pool(name="inp", bufs=2) as ipool,
            tc.tile_pool(name="outp", bufs=2) as opool,
            tc.tile_pool(name="ps", bufs=8, space="PSUM") as pspool,
        ):
            mt_sb = cpool.tile([F, F], mybir.dt.float32)
            nc.sync.dma_start(out=mt_sb[:], in_=mt[:, :])
            for c in range(n_chunks):
                it = ipool.tile([F, ch], mybir.dt.float32, tag="in")
                nc.sync.dma_start(out=it[:], in_=xt[:, c * ch:(c + 1) * ch])
                osb = opool.tile([F, ch], mybir.dt.float32, tag="out")
                for j in range(ch // NMM):
                    ps = pspool.tile([F, NMM], mybir.dt.float32, tag="ps")
                    nc.tensor.matmul(
                        ps[:],
                        mt_sb[:],
                        it[:, j * NMM:(j + 1) * NMM],
                        start=True,
                        stop=True,
                    )
                    nc.vector.tensor_copy(
                        out=osb[:, j * NMM:(j + 1) * NMM], in_=ps[:]
                    )
                nc.scalar.dma_start(out=ot[:, c * ch:(c + 1) * ch], in_=osb[:])
    nc.compile()
    return nc


_NC_CACHE = {}


def _get_nc(r=R, ch=CH):
    key = (r, ch)
    if key not in _NC_CACHE:
        _NC_CACHE[key] = _build_nc(r, ch)
    return _NC_CACHE[key]


def compose_matrix(angles, indices_in, idx_out):
    """Compose the 8 butterfly layers into one [F, F] matrix (float64)."""
    angles = np.asarray(angles, dtype=np.float64)
    ii = np.asarray(indices_in).reshape(-1, 2)
    io = np.asarray(idx_out).reshape(-1, 2)
    M = np.eye(F, dtype=np.float64)
    for l in range(angles.shape[0]):
        c = np.cos(angles[l])
        s = np.sin(angles[l])
        A = np.eye(F, dtype=np.float64)
        A[io[:, 0], :] = 0.0
        A[io[:, 1], :] = 0.0
        A[io[:, 0], ii[:, 0]] = c
        A[io[:, 0], ii[:, 1]] = -s
        A[io[:, 1], ii[:, 0]] = s
        A[io[:, 1], ii[:, 1]] = c
        M = A @ M
    return M


def _run(data, angles, indices_in, idx_out, trace=False):
    from concourse.bass_utils import run_bass_kernel_spmd

    data = np.asarray(data)
    assert data.shape == (B, F) and data.dtype == np.float32, (
        f"unexpected data {data.shape} {data.dtype}"
    )
    M = compose_matrix(angles, indices_in, idx_out)
    # lhsT layout: matmul computes lhsT.T @ rhs, we want M @ x -> lhsT = M.T
    mt_np = np.ascontiguousarray(M.T).astype(np.float32)

    in_maps = []
    for i in range(NUM_CORES):
        shard = np.ascontiguousarray(data[i * R:(i + 1) * R].T)  # [F, R]
        in_maps.append({"xt": shard, "mt": mt_np})

    nc = _get_nc()
    res = run_bass_kernel_spmd(
        nc, in_maps, core_ids=list(range(NUM_CORES)), trace=trace
    )
    out = np.concatenate(
        [res.results[i]["ot"].T for i in range(NUM_CORES)], axis=0
    )
    return np.ascontiguousarray(out), res


def kernel(data, angles, indices_in, idx_out):
    out, _ = _run(data, angles, indices_in, idx_out, trace=False)
    return out


# revision 8
# speedup vs baseline: 1.1386x; 1.1386x over previous
"""Trainium2 Bass kernel for nn_ButterflyModule (8 stacked butterfly layers).

Math: each layer applies 64 disjoint Givens rotations over feature pairs
(gather via indices_in, scatter via idx_out). Every layer is a linear map
A_l on the 128-dim feature axis, so the module collapses into a single
128x128 matrix M = A_7 @ ... @ A_0, composed on host in float64 from the
tiny angles/index inputs. Because idx_out == indices_in (as produced by
setup_inputs), M has exactly 2 nonzeros per row: one total Givens rotation
per feature pair. The 256 MB `data` tensor is processed on-device.

Distribution: pure data-parallel over 8 NeuronCores, each handling a
[65536, 128] batch shard.

Device kernel (elementwise form — no TensorE, no PSUM): the host lays the
shard out as two tensors xa/xb [128, R/2] whose lane p holds the pair-p%64
"a"/"b" feature stream (rows split into two halves across the partition
halves). The rotation is then four per-partition-scalar elementwise ops:

    ob = (xa * cba)          (ACT copy-with-scale)
    ob = (xb * cbb) + ob     (DVE scalar_tensor_tensor)
    xa = (xa * caa)          (ACT, in place)
    xa = (xb * cab) + xa     (DVE, in place -> second output)

streamed over row-chunks with double-buffered DMA. Purely HBM-bound:
64 MB of DRAM traffic per core against ~360 GB/s.
"""

import numpy as np

B = 524288          # batch rows
F = 128             # feature dim
NPAIR = F // 2
NUM_CORES = 8
R = B // NUM_CORES  # rows per core
HALF = R // 2       # columns per packed tensor
CH = 4096           # columns per DMA chunk


def _build_nc(half=HALF, ch=CH, bufs=3):
    import concourse.bacc as bacc
    import concourse.mybir as mybir
    from concourse.tile import TileContext

    # Bacc (not raw Bass): its compile() runs move_matmul_waits_to_ldweights
    # + generate_event_semaphores, which split multi-semaphore waits down to
    # the 1-wait-per-instruction hardware limit (walrus rejects otherwise).
    nc = bacc.Bacc()
    f32 = mybir.dt.float32
    xa = nc.dram_tensor("xa", [F, half], f32, kind="ExternalInput")
    xb = nc.dram_tensor("xb", [F, half], f32, kind="ExternalInput")
    cf = nc.dram_tensor("cf", [F, 4], f32, kind="ExternalInput")
    oa = nc.dram_tensor("oa", [F, half], f32, kind="ExternalOutput")
    ob = nc.dram_tensor("ob", [F, half], f32, kind="ExternalOutput")

    n_chunks = half // ch
    assert n_chunks * ch == half

    Copy = mybir.ActivationFunctionType.Copy
    mult = mybir.AluOpType.mult
    add = mybir.AluOpType.add

    with TileContext(nc) as tc:
        with (
            tc.tile_pool(name="consts", bufs=1) as cpool,
            tc.tile_pool(name="pa", bufs=bufs) as apool,
            tc.tile_pool(name="pb", bufs=bufs) as bpool,
            tc.tile_pool(name="po", bufs=2) as opool,
        ):
            cf_sb = cpool.tile([F, 4], f32)
            nc.sync.dma_start(out=cf_sb[:], in_=cf[:, :])
            caa, cab = cf_sb[:, 0:1], cf_sb[:, 1:2]
            cba, cbb = cf_sb[:, 2:3], cf_sb[:, 3:4]
            for c in range(n_chunks):
                sl = slice(c * ch, (c + 1) * ch)
                ta = apool.tile([F, ch], f32, tag="a")
                tb = bpool.tile([F, ch], f32, tag="b")
                nc.sync.dma_start(out=ta[:], in_=xa[:, sl])
                nc.sync.dma_start(out=tb[:], in_=xb[:, sl])
                to = opool.tile([F, ch], f32, tag="o")
                # ob stream into its own tile
                nc.scalar.activation(to[:], ta[:], Copy, scale=cba)
                nc.vector.scalar_tensor_tensor(
                    to[:], tb[:], cbb, to[:], op0=mult, op1=add
                )
                # oa stream computed in place over the a-tile
                nc.scalar.activation(ta[:], ta[:], Copy, scale=caa)
                nc.vector.scalar_tensor_tensor(
                    ta[:], tb[:], cab, ta[:], op0=mult, op1=add
                )
                nc.scalar.dma_start(out=ob[:, sl], in_=to[:])
                nc.scalar.dma_start(out=oa[:, sl], in_=ta[:])
    nc.compile()
    return nc


_NC_CACHE = {}


def _get_nc(key=None):
    if key not in _NC_CACHE:
        _NC_CACHE[key] = _build_nc()
    return _NC_CACHE[key]


def compose_matrix(angles, indices_in, idx_out):
    """Compose the butterfly layers into one [F, F] matrix (float64)."""
    angles = np.asarray(angles, dtype=np.float64)
    ii = np.asarray(indices_in).reshape(-1, 2)
    io = np.asarray(idx_out).reshape(-1, 2)
    M = np.eye(F, dtype=np.float64)
    for l in range(angles.shape[0]):
        c = np.cos(angles[l])
        s = np.sin(angles[l])
        A = np.eye(F, dtype=np.float64)
        A[io[:, 0], :] = 0.0
        A[io[:, 1], :] = 0.0
        A[io[:, 0], ii[:, 0]] = c
        A[io[:, 0], ii[:, 1]] = -s
        A[io[:, 1], ii[:, 0]] = s
        A[io[:, 1], ii[:, 1]] = c
        M = A @ M
    return M


def _pair_coefficients(M, indices_in):
    """Extract per-pair 2x2 rotation blocks from M.

    Returns cf [F, 4] float32 with lane p holding (caa, cab, cba, cbb) of
    pair p % 64, or None if M is not pair-block structured (cannot happen
    for inputs produced by setup_inputs, where idx_out == indices_in).
    """
    ii = np.asarray(indices_in).reshape(-1, 2)
    ia, ib = ii[:, 0], ii[:, 1]
    mask = np.zeros((F, F), dtype=bool)
    mask[ia, ia] = mask[ia, ib] = mask[ib, ia] = mask[ib, ib] = True
    if np.any(M[~mask] != 0.0):
        return None
    quad = np.stack(
        [M[ia, ia], M[ia, ib], M[ib, ia], M[ib, ib]], axis=1
    )  # [64, 4]
    return np.ascontiguousarray(np.tile(quad, (2, 1))).astype(np.float32)


def _run(data, angles, indices_in, idx_out, trace=False):
    from concourse.bass_utils import run_bass_kernel_spmd

    data = np.asarray(data)
    assert data.shape == (B, F) and data.dtype == np.float32, (
        f"unexpected data {data.shape} {data.dtype}"
    )
    M = compose_matrix(angles, indices_in, idx_out)
    cf = _pair_coefficients(M, indices_in)
    assert cf is not None, "M is not pair-structured; unexpected inputs"

    ii = np.asarray(indices_in).reshape(-1, 2)
    ia, ib = ii[:, 0], ii[:, 1]

    # Host layout: per core, gather the a/b feature streams and split the
    # row range across partition halves -> xa/xb [128, R/2].
    xa_all = np.ascontiguousarray(data[:, ia].T)  # [64, B]
    xb_all = np.ascontiguousarray(data[:, ib].T)
    in_maps = []
    for i in range(NUM_CORES):
        r0 = i * R
        xa_i = np.concatenate(
            [xa_all[:, r0:r0 + HALF], xa_all[:, r0 + HALF:r0 + R]], axis=0
        )
        xb_i = np.concatenate(
            [xb_all[:, r0:r0 + HALF], xb_all[:, r0 + HALF:r0 + R]], axis=0
        )
        in_maps.append({"xa": np.ascontiguousarray(xa_i),
                        "xb": np.ascontiguousarray(xb_i),
                        "cf": cf})

    nc = _get_nc()
    res = run_bass_kernel_spmd(
        nc, in_maps, core_ids=list(range(NUM_CORES)), trace=trace
    )

    out = np.empty((B, F), dtype=np.float32)
    for i in range(NUM_CORES):
        r0 = i * R
        ra = res.results[i]["oa"]  # [128, HALF]
        rb = res.results[i]["ob"]
        out[r0:r0 + HALF, ia] = ra[:NPAIR].T
        out[r0 + HALF:r0 + R, ia] = ra[NPAIR:].T
        out[r0:r0 + HALF, ib] = rb[:NPAIR].T
        out[r0 + HALF:r0 + R, ib] = rb[NPAIR:].T
    return out, res


def kernel(data, angles, indices_in, idx_out):
    out, _ = _run(data, angles, indices_in, idx_out, trace=False)
    return out


# revision 10
# speedup vs baseline: 1.1879x; 1.0433x over previous
"""Trainium2 Bass kernel for nn_ButterflyModule (8 stacked butterfly layers).

Math: each layer applies 64 disjoint Givens rotations over feature pairs
(gather via indices_in, scatter via idx_out). Every layer is a linear map
A_l on the 128-dim feature axis, so the module collapses into a single
128x128 matrix M = A_7 @ ... @ A_0, composed on host in float64 from the
tiny angles/index inputs. Because idx_out == indices_in (as produced by
setup_inputs), M has exactly 2 nonzeros per row: one total Givens rotation
per feature pair. The 256 MB `data` tensor is processed on-device.

Distribution: pure data-parallel over 8 NeuronCores, each handling a
[65536, 128] batch shard.

Device kernel (elementwise form — no TensorE, no PSUM): the host lays the
shard out as two tensors xa/xb [128, R/2] whose lane p holds the pair-p%64
"a"/"b" feature stream (rows split into two halves across the partition
halves). The rotation is then four per-partition-scalar elementwise ops:

    ob = (xa * cba)          (ACT copy-with-scale)
    ob = (xb * cbb) + ob     (DVE scalar_tensor_tensor)
    xa = (xa * caa)          (ACT, in place)
    xa = (xb * cab) + xa     (DVE, in place -> second output)

streamed over row-chunks with double-buffered DMA. Purely HBM-bound:
64 MB of DRAM traffic per core against ~360 GB/s.
"""

import numpy as np

B = 524288          # batch rows
F = 128             # feature dim
NPAIR = F // 2
NUM_CORES = 8
R = B // NUM_CORES  # rows per core
HALF = R // 2       # columns per packed tensor
CH = 4096           # columns per DMA chunk


def _chunk_schedule(half, ch):
    """Chunk sizes summing to `half`: small chunks at the head (faster
    pipeline ramp-up — compute starts after the first small DMA instead of
    a full-size one) and at the tail (shorter post-compute DMA drain)."""
    ramp = [ch // 4, ch // 4, ch // 2]
    body = half - 2 * sum(ramp)
    assert body >= 0 and body % ch == 0
    return ramp + [ch] * (body // ch) + ramp[::-1]


def _build_nc(half=HALF, ch=CH, bufs=3, ramp=True):
    import concourse.bacc as bacc
    import concourse.mybir as mybir
    from concourse.tile import TileContext

    # Bacc (not raw Bass): its compile() runs move_matmul_waits_to_ldweights
    # + generate_event_semaphores, which split multi-semaphore waits down to
    # the 1-wait-per-instruction hardware limit (walrus rejects otherwise).
    nc = bacc.Bacc()
    f32 = mybir.dt.float32
    xa = nc.dram_tensor("xa", [F, half], f32, kind="ExternalInput")
    xb = nc.dram_tensor("xb", [F, half], f32, kind="ExternalInput")
    cf = nc.dram_tensor("cf", [F, 4], f32, kind="ExternalInput")
    oa = nc.dram_tensor("oa", [F, half], f32, kind="ExternalOutput")
    ob = nc.dram_tensor("ob", [F, half], f32, kind="ExternalOutput")

    chunks = _chunk_schedule(half, ch) if ramp else [ch] * (half // ch)
    assert sum(chunks) == half

    Copy = mybir.ActivationFunctionType.Copy
    mult = mybir.AluOpType.mult
    add = mybir.AluOpType.add

    with TileContext(nc) as tc:
        with (
            tc.tile_pool(name="consts", bufs=1) as cpool,
            tc.tile_pool(name="pa", bufs=bufs) as apool,
            tc.tile_pool(name="pb", bufs=bufs) as bpool,
            tc.tile_pool(name="po", bufs=2) as opool,
        ):
            cf_sb = cpool.tile([F, 4], f32)
            nc.sync.dma_start(out=cf_sb[:], in_=cf[:, :])
            caa, cab = cf_sb[:, 0:1], cf_sb[:, 1:2]
            cba, cbb = cf_sb[:, 2:3], cf_sb[:, 3:4]
            pos = 0
            for csz in chunks:
                sl = slice(pos, pos + csz)
                pos += csz
                ta_full = apool.tile([F, ch], f32, tag="a")
                tb_full = bpool.tile([F, ch], f32, tag="b")
                to_full = opool.tile([F, ch], f32, tag="o")
                ta, tb, to = ta_full[:, :csz], tb_full[:, :csz], to_full[:, :csz]
                nc.sync.dma_start(out=ta, in_=xa[:, sl])
                nc.sync.dma_start(out=tb, in_=xb[:, sl])
                # ob stream into its own tile
                nc.scalar.activation(to, ta, Copy, scale=cba)
                nc.vector.scalar_tensor_tensor(
                    to, tb, cbb, to, op0=mult, op1=add
                )
                # oa stream computed in place over the a-tile
                nc.scalar.activation(ta, ta, Copy, scale=caa)
                nc.vector.scalar_tensor_tensor(
                    ta, tb, cab, ta, op0=mult, op1=add
                )
                nc.scalar.dma_start(out=ob[:, sl], in_=to)
                nc.scalar.dma_start(out=oa[:, sl], in_=ta)
    nc.compile()
    return nc


_NC_CACHE = {}


def _get_nc(key=None):
    if key not in _NC_CACHE:
        _NC_CACHE[key] = _build_nc()
    return _NC_CACHE[key]


def compose_matrix(angles, indices_in, idx_out):
    """Compose the butterfly layers into one [F, F] matrix (float64)."""
    angles = np.asarray(angles, dtype=np.float64)
    ii = np.asarray(indices_in).reshape(-1, 2)
    io = np.asarray(idx_out).reshape(-1, 2)
    M = np.eye(F, dtype=np.float64)
    for l in range(angles.shape[0]):
        c = np.cos(angles[l])
        s = np.sin(angles[l])
        A = np.eye(F, dtype=np.float64)
        A[io[:, 0], :] = 0.0
        A[io[:, 1], :] = 0.0
        A[io[:, 0], ii[:, 0]] = c
        A[io[:, 0], ii[:, 1]] = -s
        A[io[:, 1], ii[:, 0]] = s
        A[io[:, 1], ii[:, 1]] = c
        M = A @ M
    return M


def _pair_coefficients(M, indices_in):
    """Extract per-pair 2x2 rotation blocks from M.

    Returns cf [F, 4] float32 with lane p holding (caa, cab, cba, cbb) of
    pair p % 64, or None if M is not pair-block structured (cannot happen
    for inputs produced by setup_inputs, where idx_out == indices_in).
    """
    ii = np.asarray(indices_in).reshape(-1, 2)
    ia, ib = ii[:, 0], ii[:, 1]
    mask = np.zeros((F, F), dtype=bool)
    mask[ia, ia] = mask[ia, ib] = mask[ib, ia] = mask[ib, ib] = True
    if np.any(M[~mask] != 0.0):
        return None
    quad = np.stack(
        [M[ia, ia], M[ia, ib], M[ib, ia], M[ib, ib]], axis=1
    )  # [64, 4]
    return np.ascontiguousarray(np.tile(quad, (2, 1))).astype(np.float32)


def _run(data, angles, indices_in, idx_out, trace=False):
    from concourse.bass_utils import run_bass_kernel_spmd

    data = np.asarray(data)
    assert data.shape == (B, F) and data.dtype == np.float32, (
        f"unexpected data {data.shape} {data.dtype}"
    )
    M = compose_matrix(angles, indices_in, idx_out)
    cf = _pair_coefficients(M, indices_in)
    assert cf is not None, "M is not pair-structured; unexpected inputs"

    ii = np.asarray(indices_in).reshape(-1, 2)
    ia, ib = ii[:, 0], ii[:, 1]

    # Host layout: per core, gather the a/b feature streams and split the
    # row range across partition halves -> xa/xb [128, R/2].
    xa_all = np.ascontiguousarray(data[:, ia].T)  # [64, B]
    xb_all = np.ascontiguousarray(data[:, ib].T)
    in_maps = []
    for i in range(NUM_CORES):
        r0 = i * R
        xa_i = np.concatenate(
            [xa_all[:, r0:r0 + HALF], xa_all[:, r0 + HALF:r0 + R]], axis=0
        )
        xb_i = np.concatenate(
            [xb_all[:, r0:r0 + HALF], xb_all[:, r0 + HALF:r0 + R]], axis=0
        )
        in_maps.append({"xa": np.ascontiguousarray(xa_i),
                        "xb": np.ascontiguousarray(xb_i),
                        "cf": cf})

    nc = _get_nc()
    res = run_bass_kernel_spmd(
        nc, in_maps, core_ids=list(range(NUM_CORES)), trace=trace
    )

    out = np.empty((B, F), dtype=np.float32)
    for i in range(NUM_CORES):
        r0 = i * R
        ra = res.results[i]["oa"]  # [128, HALF]
        rb = res.results[i]["ob"]
        out[r0:r0 + HALF, ia] = ra[:NPAIR].T
        out[r0 + HALF:r0 + R, ia] = ra[NPAIR:].T
        out[r0:r0 + HALF, ib] = rb[:NPAIR].T
        out[r0 + HALF:r0 + R, ib] = rb[NPAIR:].T
    return out, res


def kernel(data, angles, indices_in, idx_out):
    out, _ = _run(data, angles, indices_in, idx_out, trace=False)
    return out
